# revision 29
# baseline (speedup 1.0000x reference)
"""BilinearInteraction Trainium2 kernel (8 NeuronCores, batch-sharded).

out[b, p=(i,j), d] = x[b, i, d] * (x @ W)[b, j, d]  for the 496 upper-tri
pairs of F=32 fields; x [4096, 32, 64] f32, W [64, 64] f32.

bf16 end-to-end (the correctness gate is rel_err < 2e-2; this pipeline
lands at ~5.5e-3). bf16 halves BOTH limiting resources vs f32: the DVE
tensor_tensor multiply runs in 2x_1P perf mode (f32 runs 1x), and the
32.5 MB-per-core output store stream fits the ~400-430 GB/s per-core
HBM write rate in ~80 us. In steady state the DVE multiply stream
(~85 us busy) and the store stream are nearly perfectly balanced.

Per core: 512 batch rows, processed as 4 tiles of 128 (batch on SBUF
partitions). Per tile, in descending field order so the first-processed
output chunk only needs the tail of vid:
  - vid = x @ W via PE pair-block transposes ([128,128] f-pair blocks
    -> PSUM bf16) + bf16 matmuls against a host-provided block-diag
    [[W,0],[0,W]] (two fields per instruction), grouped 4-to-a-PSUM-bank
    so ACT moves PSUM->SBUF (casting f32->bf16) in few fat copies.
  - pairwise Hadamard on DVE: one tensor_mul per field i covering all
    j>i at once, broadcasting x[:,i,:] over the j axis with a stride-0
    access pattern (the broadcast does not break 2x mode).
  - output staged in SBUF in 6 block-aligned chunks, each DMA'd as one
    1.2-1.5 MB contiguous-per-partition store on the sync HWDGE ring
    (fat stores sustain 400+ GB/s; per-op small stores measured ~350);
    inputs ride the scalar-engine ring so they never queue behind
    output stores.
Ramp: tile 0's x loads high-fields-first (the first-processed chunk
only reads fields >=19) and the first chunk's store is split in three,
so the output stream starts ~16 us into the kernel.
Host converts x/W to bf16 on the way in, result back to f32 on the out.
"""

import sys

if "/opt/trn_rl_repo" not in sys.path:
    sys.path.insert(0, "/opt/trn_rl_repo")

import numpy as np
import ml_dtypes

import concourse.bass as bass
import concourse.mybir as mybir
import concourse.tile as tile
from concourse import bacc
from concourse.bass_utils import run_bass_kernel_spmd

B, F, D = 4096, 32, 64
P = F * (F - 1) // 2
NCORES = 8
BSH = B // NCORES
BT = 128
NTILES = BSH // BT
FD = F * D

bf16 = mybir.dt.bfloat16
f32 = mybir.dt.float32
np_bf16 = ml_dtypes.bfloat16

POFF = [0]
for i in range(F - 1):
    POFF.append(POFF[-1] + (F - 1 - i))
CHUNKS = [(0, 90), (90, 171), (171, 265), (265, 343), (343, 418), (418, 496)]


def _emit(tc, nc, x_d, w2_d, i128_d, out_d):
    with (
        tc.tile_pool(name="const", bufs=1) as const_pool,
        tc.tile_pool(name="xp", bufs=4) as x_pool,
        tc.tile_pool(name="vidp", bufs=2) as vid_pool,
        tc.tile_pool(name="xtp", bufs=4) as xt_pool,
        tc.tile_pool(name="outp", bufs=6) as out_pool,
        tc.tile_pool(name="ps_t", bufs=2, space="PSUM") as ps_t,
        tc.tile_pool(name="ps_m", bufs=2, space="PSUM") as ps_m,
    ):
        x_ts = []
        for t in range(NTILES):
            x_t = x_pool.tile([128, FD], bf16, tag="xt")
            x_ts.append(x_t)
        nc.scalar.dma_start(
            out=x_ts[0][:, FD // 2 :].rearrange("p (f d) -> p f d", d=D),
            in_=x_d[0:BT, F // 2 :, :],
        )
        ident = const_pool.tile([128, 128], bf16)
        nc.scalar.dma_start(out=ident[:], in_=i128_d[:])
        w2 = const_pool.tile([128, 128], bf16)
        nc.scalar.dma_start(out=w2[:], in_=w2_d[:])
        nc.scalar.dma_start(
            out=x_ts[0][:, : FD // 2].rearrange("p (f d) -> p f d", d=D),
            in_=x_d[0:BT, : F // 2, :],
        )
        for t in range(1, NTILES):
            nc.scalar.dma_start(
                out=x_ts[t][:].rearrange("p (f d) -> p f d", d=D),
                in_=x_d[t * BT : (t + 1) * BT, :, :],
            )

        for t in range(NTILES):
            b0 = t * BT
            x_t = x_ts[t]
            x3 = x_t[:].rearrange("p (f d) -> p f d", d=D)

            vid_t = vid_pool.tile([128, FD], bf16, tag="vidt")
            for g in reversed(range(4)):
                xT_ps = ps_t.tile([128, 512], bf16, tag="xtps")
                for k in range(4):
                    nc.tensor.transpose(
                        xT_ps[:, k * 128 : (k + 1) * 128],
                        x_t[:, (4 * g + k) * 128 : (4 * g + k + 1) * 128],
                        ident[:],
                    )
                xT_sb = xt_pool.tile([128, 512], bf16, tag="xtsb")
                nc.scalar.copy(xT_sb[:], xT_ps[:])
                vid_ps = ps_m.tile([128, 512], f32, tag="vidps")
                for k in range(4):
                    nc.tensor.matmul(
                        vid_ps[:, k * 128 : (k + 1) * 128],
                        xT_sb[:, k * 128 : (k + 1) * 128],
                        w2[:],
                        start=True,
                        stop=True,
                    )
                nc.scalar.copy(vid_t[:, g * 512 : (g + 1) * 512], vid_ps[:])
            vid3 = vid_t[:].rearrange("p (f d) -> p f d", d=D)

            for ci, (c0, c1) in enumerate(reversed(CHUNKS)):
                npair = c1 - c0
                o_t = out_pool.tile([128, npair * D], bf16, tag="outs")
                o3 = o_t[:].rearrange("p (q d) -> p q d", d=D)
                for i in reversed(range(F - 1)):
                    blk0, blk1 = POFF[i], POFF[i + 1]
                    lo, hi = max(blk0, c0), min(blk1, c1)
                    if lo >= hi:
                        continue
                    nj = hi - lo
                    j0 = i + 1 + (lo - blk0)
                    nc.vector.tensor_mul(
                        o3[:, lo - c0 : hi - c0, :],
                        x3[:, i : i + 1, :].broadcast_to((128, nj, D)),
                        vid3[:, j0 : j0 + nj, :],
                    )
                if t == 0 and ci == 0:
                    subs = ((475, 496), (451, 475), (418, 451))
                else:
                    subs = ((c0, c1),)
                for s0, s1 in subs:
                    nc.sync.dma_start(
                        out=out_d[b0 : b0 + BT, s0:s1, :],
                        in_=o3[:, s0 - c0 : s1 - c0, :],
                    )


def build_nc():
    nc = bacc.Bacc("TRN2", target_bir_lowering=False, debug=False)
    x_d = nc.dram_tensor("x", [BSH, F, D], bf16, kind="ExternalInput")
    w2_d = nc.dram_tensor("W2", [128, 128], bf16, kind="ExternalInput")
    i128_d = nc.dram_tensor("I128", [128, 128], bf16, kind="ExternalInput")
    out_d = nc.dram_tensor("out", [BSH, P, D], bf16, kind="ExternalOutput")
    with tile.TileContext(nc) as tc:
        _emit(tc, nc, x_d.ap(), w2_d.ap(), i128_d.ap(), out_d.ap())
    nc.compile()
    return nc


_NC = None


def kernel(x: np.ndarray, W: np.ndarray, _trace=False, _trace_kwargs=None):
    global _NC
    if _NC is None:
        _NC = build_nc()
    x16 = np.ascontiguousarray(x, dtype=np.float32).astype(np_bf16)
    W = np.ascontiguousarray(W, dtype=np.float32)
    w2 = np.zeros((128, 128), dtype=np.float32)
    w2[:64, :64] = W
    w2[64:, 64:] = W
    w2 = w2.astype(np_bf16)
    i128 = np.eye(128, dtype=np_bf16)
    in_maps = [
        {"x": x16[i * BSH : (i + 1) * BSH], "W2": w2, "I128": i128}
        for i in range(NCORES)
    ]
    res = run_bass_kernel_spmd(
        _NC,
        in_maps,
        core_ids=list(range(NCORES)),
        trace=_trace,
        **(_trace_kwargs or {}),
    )
    out = np.concatenate(
        [res.results[i]["out"].astype(np.float32) for i in range(NCORES)], axis=0
    )
    if _trace:
        return out, res
    return out


# revision 30
# speedup vs baseline: 1.1536x; 1.1536x over previous
"""BilinearInteraction Trainium2 kernel (8 NeuronCores, batch-sharded).

out[b, p=(i,j), d] = x[b, i, d] * (x @ W)[b, j, d]  for the 496 upper-tri
pairs of F=32 fields; x [4096, 32, 64] f32, W [64, 64] f32.

bf16 end-to-end (the correctness gate is rel_err < 2e-2; this pipeline
lands at ~5.5e-3). bf16 halves BOTH limiting resources vs f32: the DVE
tensor_tensor multiply runs in 2x_1P perf mode (f32 runs 1x), and the
32.5 MB-per-core output store stream fits the ~400-430 GB/s per-core
HBM write rate in ~80 us. In steady state the DVE multiply stream
(~85 us busy) and the store stream are nearly perfectly balanced.

Per core: 512 batch rows, processed as 4 tiles of 128 (batch on SBUF
partitions). Per tile, in descending field order so the first-processed
output chunk only needs the tail of vid:
  - vid = x @ W via PE pair-block transposes ([128,128] f-pair blocks
    -> PSUM bf16) + bf16 matmuls against a host-provided block-diag
    [[W,0],[0,W]] (two fields per instruction), grouped 4-to-a-PSUM-bank
    so ACT moves PSUM->SBUF (casting f32->bf16) in few fat copies.
  - pairwise Hadamard on DVE: one tensor_mul per field i covering all
    j>i at once, broadcasting x[:,i,:] over the j axis with a stride-0
    access pattern (the broadcast does not break 2x mode).
  - output staged in SBUF in 6 block-aligned chunks, each DMA'd as one
    1.2-1.5 MB contiguous-per-partition store on the sync HWDGE ring
    (fat stores sustain 400+ GB/s; per-op small stores measured ~350);
    inputs ride the scalar-engine ring so they never queue behind
    output stores.
Ramp: tile 0's x loads high-fields-first (the first-processed chunk
only reads fields >=19) and the first chunk's store is split in three,
so the output stream starts ~16 us into the kernel.
Host converts x/W to bf16 on the way in, result back to f32 on the out.
"""

import sys

if "/opt/trn_rl_repo" not in sys.path:
    sys.path.insert(0, "/opt/trn_rl_repo")

import numpy as np
import ml_dtypes

import concourse.bass as bass
import concourse.mybir as mybir
import concourse.tile as tile
from concourse import bacc
from concourse.bass_utils import run_bass_kernel_spmd

B, F, D = 4096, 32, 64
P = F * (F - 1) // 2
NCORES = 8
BSH = B // NCORES
BT = 128
NTILES = BSH // BT
FD = F * D

bf16 = mybir.dt.bfloat16
f32 = mybir.dt.float32
np_bf16 = ml_dtypes.bfloat16

POFF = [0]
for i in range(F - 1):
    POFF.append(POFF[-1] + (F - 1 - i))
CHUNKS = [(0, 90), (90, 171), (171, 265), (265, 343), (343, 418), (418, 496)]


def _emit(tc, nc, x_d, w2_d, i128_d, out_d):
    with (
        tc.tile_pool(name="const", bufs=1) as const_pool,
        tc.tile_pool(name="xp", bufs=4) as x_pool,
        tc.tile_pool(name="vidp", bufs=2) as vid_pool,
        tc.tile_pool(name="xtp", bufs=4) as xt_pool,
        tc.tile_pool(name="outp", bufs=6) as out_pool,
        tc.tile_pool(name="ps_t", bufs=2, space="PSUM") as ps_t,
        tc.tile_pool(name="ps_m", bufs=2, space="PSUM") as ps_m,
    ):
        x_ts = []
        for t in range(NTILES):
            x_t = x_pool.tile([128, FD], bf16, tag="xt")
            x_ts.append(x_t)
        nc.scalar.dma_start(
            out=x_ts[0][:, FD // 2 :].rearrange("p (f d) -> p f d", d=D),
            in_=x_d[0:BT, F // 2 :, :],
        )
        ident = const_pool.tile([128, 128], bf16)
        nc.scalar.dma_start(out=ident[:], in_=i128_d[:])
        w2 = const_pool.tile([128, 128], bf16)
        nc.scalar.dma_start(out=w2[:], in_=w2_d[:])
        nc.scalar.dma_start(
            out=x_ts[0][:, : FD // 2].rearrange("p (f d) -> p f d", d=D),
            in_=x_d[0:BT, : F // 2, :],
        )
        for t in range(1, NTILES):
            nc.scalar.dma_start(
                out=x_ts[t][:].rearrange("p (f d) -> p f d", d=D),
                in_=x_d[t * BT : (t + 1) * BT, :, :],
            )

        for t in range(NTILES):
            b0 = t * BT
            x_t = x_ts[t]
            x3 = x_t[:].rearrange("p (f d) -> p f d", d=D)

            vid_t = vid_pool.tile([128, FD], bf16, tag="vidt")
            for g in reversed(range(4)):
                xT_ps = ps_t.tile([128, 512], bf16, tag="xtps")
                for k in range(4):
                    nc.tensor.transpose(
                        xT_ps[:, k * 128 : (k + 1) * 128],
                        x_t[:, (4 * g + k) * 128 : (4 * g + k + 1) * 128],
                        ident[:],
                    )
                xT_sb = xt_pool.tile([128, 512], bf16, tag="xtsb")
                nc.scalar.copy(xT_sb[:], xT_ps[:])
                vid_ps = ps_m.tile([128, 512], f32, tag="vidps")
                for k in range(4):
                    nc.tensor.matmul(
                        vid_ps[:, k * 128 : (k + 1) * 128],
                        xT_sb[:, k * 128 : (k + 1) * 128],
                        w2[:],
                        start=True,
                        stop=True,
                    )
                nc.scalar.copy(vid_t[:, g * 512 : (g + 1) * 512], vid_ps[:])
            vid3 = vid_t[:].rearrange("p (f d) -> p f d", d=D)

            for ci, (c0, c1) in enumerate(reversed(CHUNKS)):
                npair = c1 - c0
                o_t = out_pool.tile([128, npair * D], bf16, tag="outs")
                o3 = o_t[:].rearrange("p (q d) -> p q d", d=D)
                for i in reversed(range(F - 1)):
                    blk0, blk1 = POFF[i], POFF[i + 1]
                    lo, hi = max(blk0, c0), min(blk1, c1)
                    if lo >= hi:
                        continue
                    nj = hi - lo
                    j0 = i + 1 + (lo - blk0)
                    nc.vector.tensor_mul(
                        o3[:, lo - c0 : hi - c0, :],
                        x3[:, i : i + 1, :].broadcast_to((128, nj, D)),
                        vid3[:, j0 : j0 + nj, :],
                    )
                if t == 0 and ci == 0:
                    # per-block sub-stores: the first store fires after a
                    # single TT (~0.2 us) instead of seven (~2.3 us); the
                    # DMA queue is idle during the ramp, so small
                    # descriptors cost nothing here
                    subs = ((495, 496), (493, 495), (490, 493), (486, 490),
                            (481, 486), (475, 481), (468, 475), (451, 468),
                            (418, 451))
                else:
                    subs = ((c0, c1),)
                for s0, s1 in subs:
                    nc.sync.dma_start(
                        out=out_d[b0 : b0 + BT, s0:s1, :],
                        in_=o3[:, s0 - c0 : s1 - c0, :],
                    )


def build_nc():
    nc = bacc.Bacc("TRN2", target_bir_lowering=False, debug=False)
    x_d = nc.dram_tensor("x", [BSH, F, D], bf16, kind="ExternalInput")
    w2_d = nc.dram_tensor("W2", [128, 128], bf16, kind="ExternalInput")
    i128_d = nc.dram_tensor("I128", [128, 128], bf16, kind="ExternalInput")
    out_d = nc.dram_tensor("out", [BSH, P, D], bf16, kind="ExternalOutput")
    with tile.TileContext(nc) as tc:
        _emit(tc, nc, x_d.ap(), w2_d.ap(), i128_d.ap(), out_d.ap())
    nc.compile()
    return nc


_NC = None


def kernel(x: np.ndarray, W: np.ndarray, _trace=False, _trace_kwargs=None):
    global _NC
    if _NC is None:
        _NC = build_nc()
    x16 = np.ascontiguousarray(x, dtype=np.float32).astype(np_bf16)
    W = np.ascontiguousarray(W, dtype=np.float32)
    w2 = np.zeros((128, 128), dtype=np.float32)
    w2[:64, :64] = W
    w2[64:, 64:] = W
    w2 = w2.astype(np_bf16)
    i128 = np.eye(128, dtype=np_bf16)
    in_maps = [
        {"x": x16[i * BSH : (i + 1) * BSH], "W2": w2, "I128": i128}
        for i in range(NCORES)
    ]
    res = run_bass_kernel_spmd(
        _NC,
        in_maps,
        core_ids=list(range(NCORES)),
        trace=_trace,
        **(_trace_kwargs or {}),
    )
    out = np.concatenate(
        [res.results[i]["out"].astype(np.float32) for i in range(NCORES)], axis=0
    )
    if _trace:
        return out, res
    return out
